# revision 22
# baseline (speedup 1.0000x reference)
"""CenterCut2 Trainium2 kernel (v2 — software-pipelined).

For each sample b: find argmax of power = sum_c x[b,c]^2 over the (D,H,W)
volume, then extract the 16x32x32 window centered on the peak with circular
wraparound (equivalent to reference's per-sample roll + center crop).

Sharding: pure data parallelism, 4 samples per core across 8 cores.

Per-core device program (samples s=0..3, volumes v=2s+c laid out [128, 8192]
with flat voxel index = p*8192 + f = dd*16384 + hh*128 + ww):

  Streaming (per 2MiB chunk pair): DMA x0,x1 chunks; ACT squares in place;
  DVE tensor_tensor_reduce fuses the power add with a running per-partition
  max (kills the separate MAX8 scan).

  Argmax: one FIND_INDEX8 scan (in_max = broadcast per-partition max), then
  the global lowest-flat-index tie-break via two gpsimd partition_all_reduce
  ops with the BIG-constant trick.  The index-decode chain and the gather
  index build run entirely on gpsimd (off the DVE critical path), 128-wide
  where possible.

  Gather: one 96-descriptor dma_gather at 16-h-row granularity (8KB per
  descriptor) pulls, for each of 16 d-slices and 2 channels, the three
  16-row h-blocks covering the 32-row h-window; block k lands at partitions
  k*32 + c*16 + i.

  Extract (emitted one sample late so it overlaps the next sample's
  stream/scan): three partition-shifted merge copies build a [32, 48, 160]
  w-doubled tile (split ACT/DVE), then a register-offset (bass.ds) strided
  copy on ACT selects the [32h x 32w] window; one [32, 1024] DMA writes the
  sample's output rows.
"""
import sys

sys.path.insert(0, "/opt/trn_rl_repo")

import numpy as np

import concourse.bass as bass
import concourse.bacc as bacc
import concourse.mybir as mybir
from concourse.tile import TileContext
from concourse.tile_rust import add_dep_helper
from concourse.bass_utils import run_bass_kernel_spmd
from concourse.bass_isa import ReduceOp

F32 = mybir.dt.float32
I32 = mybir.dt.int32
I16 = mybir.dt.int16
U32 = mybir.dt.uint32
A = mybir.AluOpType
ACT_E = mybir.EngineType.Activation

N_CORES = 8
S_PER_CORE = 4          # samples per core
N_VOLS = 2 * S_PER_CORE # channel volumes per core
VOL = 64 * 128 * 128    # voxels per volume
FREE = VOL // 128       # 8192 free elements per partition
CHUNK = 4096            # streaming chunk (2 MiB per DMA)
BIG = float(1 << 21)

_cache = {}


def _build():
    nc = bacc.Bacc("TRN2", target_bir_lowering=False, debug=False, num_devices=N_CORES)
    x = nc.dram_tensor("x", [N_VOLS, 128, FREE], F32, kind="ExternalInput")
    y = nc.dram_tensor("y", [128, 1024], F32, kind="ExternalOutput")

    iota_base_c = nc.inline_tensor(
        (np.arange(128, dtype=np.float32) * FREE).reshape(128, 1), name="iota_base"
    )
    iota16_c = nc.inline_tensor(
        (np.arange(128, dtype=np.int32) % 16).reshape(128, 1), name="iota16"
    )
    # gather source view: [4096 rows, 2048] — row = vol*512 + dd*8 + hblk
    # (hblk = hh//16 in [0,8); each row = 16 h-rows x 128 w, 8KB)
    xrows = x.ap().rearrange("v p (a b) -> (v p a) b", a=4)

    with TileContext(nc) as tc:
        with (
            tc.tile_pool(name="xc", bufs=4) as xpool,
            tc.tile_pool(name="pw", bufs=2) as ppool,
            tc.tile_pool(name="gt", bufs=2) as gpool,
            tc.tile_pool(name="wt", bufs=1) as wpool,
            tc.tile_pool(name="ob", bufs=2) as opool,
            tc.tile_pool(name="sm", bufs=2) as spool,
            tc.tile_pool(name="big", bufs=1) as bpool,
        ):
            base = bpool.tile([128, 1], F32, tag="base")
            nc.sync.dma_start(base[:, :], iota_base_c.ap()[:, :])
            iota16 = bpool.tile([128, 1], I32, tag="iota16")
            nc.sync.dma_start(iota16[:, :], iota16_c.ap()[:, :])
            scal = bpool.tile([1, 64], I32, tag="scal")

            def ts(eng, dst, src, s1, s2, op0, op1=None):
                if op1 is None:
                    return eng.tensor_scalar(
                        out=dst, in0=src, scalar1=s1, scalar2=None, op0=op0
                    )
                return eng.tensor_scalar(
                    out=dst, in0=src, scalar1=s1, scalar2=s2, op0=op0, op1=op1
                )

            # Cross-partition work stays OFF gpsimd: mixing partition_all_
            # reduce/broadcast with dma_gather on the Pool engine forces a
            # ucode library swap (MODIFY_POOL_CONFIG + ~6us IRAM reload)
            # around every gather, serializing the whole tail.  Instead a
            # small column->row DMA moves per-partition values onto one
            # partition, where the global reduce is a cheap row op.
            big_c = nc.inline_tensor(
                np.full((1, 128), BIG, dtype=np.float32), name="bigrow"
            )
            bigrow = bpool.tile([1, 128], F32, tag="bigrow")
            nc.sync.dma_start(bigrow[:, :], big_c.ap()[:, :])

            pending = []  # (t, G, scal_base, s16_inst, w0_inst)

            def emit_extract(t, G, cb, s16_inst, w0_inst):
                W = wpool.tile([32, 48 * 160], F32, tag="W")
                w3 = W[:, :].rearrange("p (r w) -> p r w", w=160)
                g3 = G[:, :].rearrange("p (r w) -> p r w", w=128)
                # merge the three 16-row h-blocks (partition-shifted copies)
                nc.scalar.copy(w3[:, 0:16, 0:128], g3[0:32, :, :])
                nc.vector.tensor_copy(w3[:, 16:32, 0:128], g3[32:64, :, :])
                nc.scalar.copy(w3[:, 32:48, 0:128], g3[64:96, :, :])
                # w doubling for circular w wrap
                nc.scalar.copy(w3[:, :, 128:160], w3[:, :, 0:32])

                li_s, (s16_val,) = nc.values_load_multi_w_load_instructions(
                    scal[0:1, cb + 11 : cb + 12], engines=(ACT_E,),
                    min_val=0, max_val=16, skip_runtime_bounds_check=True,
                )
                li_w, (w0_val,) = nc.values_load_multi_w_load_instructions(
                    scal[0:1, cb + 10 : cb + 11], engines=(ACT_E,),
                    min_val=0, max_val=128, skip_runtime_bounds_check=True,
                )
                for L in li_s:
                    add_dep_helper(L.ins, s16_inst.ins, sync=True, reason="load after s16 write")
                for L in li_w:
                    add_dep_helper(L.ins, w0_inst.ins, sync=True, reason="load after w0 write")

                out_sb = opool.tile([32, 1024], F32, tag="out_sb")
                o3 = out_sb[:, :].rearrange("p (a b) -> p a b", a=32)
                sel = w3[0:32, bass.ds(s16_val, 32), bass.ds(w0_val, 32)]
                nc.scalar.copy(o3[:, :, :], sel)
                nc.sync.dma_start(y[32 * t : 32 * t + 32, :], out_sb[:, :])

            for s in range(S_PER_CORE):
                power = ppool.tile([128, FREE], F32, tag="pw")
                # finer chunks for sample 0 so compute starts ~5 us in
                # instead of waiting for a full 2 MiB chunk pair
                ck = 1024 if s == 0 else CHUNK
                tag = "xc0" if s == 0 else "xc"
                for k in range(FREE // ck):
                    sl = slice(k * ck, (k + 1) * ck)
                    x0 = xpool.tile([128, ck], F32, tag=tag)
                    nc.sync.dma_start(x0[:, :], x[2 * s, :, sl])
                    x1 = xpool.tile([128, ck], F32, tag=tag)
                    nc.sync.dma_start(x1[:, :], x[2 * s + 1, :, sl])
                    nc.scalar.square(x0[:, :], x0[:, :])
                    nc.scalar.square(x1[:, :], x1[:, :])
                    nc.vector.tensor_add(power[:, sl], x0[:, :], x1[:, :])

                # per-partition max (tensor_tensor_reduce crashes HW, so a
                # separate reduce scan); packed [pmax, flatf] for one DMA
                pk2 = spool.tile([128, 2], F32, tag="pk2")
                acc = pk2[:, 0:1]
                nc.vector.tensor_reduce(
                    out=acc, in_=power[:, :], axis=mybir.AxisListType.X, op=A.max
                )

                # per-partition argmax of power (one full DVE scan)
                idx8 = spool.tile([128, 8], U32, tag="idx8")
                nc.vector.max_index(
                    out=idx8[:, :],
                    in_max=acc.broadcast_to((128, 8)),
                    in_values=power[:, :],
                )
                nc.vector.tensor_copy(pk2[:, 1:2], idx8[:, 0:1])  # u32 -> f32
                nc.vector.tensor_add(pk2[:, 1:2], pk2[:, 1:2], base[:, :])

                # column -> row: per-partition (pmax, flat) pairs onto
                # partition 0, then the global argmax + lowest-flat
                # tie-break are 4 tiny row ops
                row = spool.tile([1, 256], F32, tag="row")
                nc.sync.dma_start(row[0:1, :], pk2[:, :])
                r3 = row[0:1, :].rearrange("p (a b) -> p a b", b=2)
                pmaxr = r3[:, :, 0]
                flatr = r3[:, :, 1]
                gmax = spool.tile([1, 1], F32, tag="gmax")
                nc.vector.tensor_reduce(
                    out=gmax[:, :], in_=pmaxr, axis=mybir.AxisListType.X, op=A.max
                )
                eqbig = spool.tile([1, 128], F32, tag="eqbig")
                nc.vector.scalar_tensor_tensor(
                    out=eqbig[:, :], in0=pmaxr, scalar=gmax[:, :], in1=bigrow[:, :],
                    op0=A.is_equal, op1=A.mult,
                )
                nc.vector.tensor_tensor(
                    out=eqbig[:, :], in0=eqbig[:, :], in1=flatr, op=A.subtract
                )
                allcand = spool.tile([1, 1], F32, tag="allcand")
                nc.vector.tensor_reduce(
                    out=allcand[:, :], in_=eqbig[:, :], axis=mybir.AxisListType.X, op=A.max
                )

                # extract for the previous sample goes here: its gather
                # finished a full sample ago, and it absorbs the row-DMA
                # wait before this sample's decode
                if len(pending) > 0:
                    emit_extract(*pending.pop(0))

                # ---- decode chain on DVE (Pool rejects tensor ops in codegen)
                # cols: 0=C0 1=d56 2..7=bvol[t] 8=h 6=h0 9=w 10=w0 11=s16
                # 12=b0 13=b1 14=b2 15=d
                # NOTE: walrus forbids mixing bitwise and arith ops in one
                # tensor_scalar, so shifts/ands and adds/mods stay separate.
                cb = 16 * s
                V = nc.vector

                def C(j):
                    return scal[:, cb + j : cb + j + 1]

                flat32 = spool.tile([1, 1], F32, tag="flat32")
                ts(V, flat32[:, :], allcand[0:1, 0:1], BIG, -1.0, A.subtract, A.mult)
                V.tensor_copy(C(0), flat32[:, :])                    # f32 -> i32
                ts(V, C(15), C(0), 14, None, A.logical_shift_right)  # d
                ts(V, C(1), C(15), 56, None, A.add)                  # d56
                ts(V, C(8), C(0), 7, 127, A.logical_shift_right, A.bitwise_and)  # h
                ts(V, C(6), C(8), 112, None, A.add)                  # h0 (pre-mask)
                ts(V, C(6), C(6), 127, None, A.bitwise_and)          # h0
                ts(V, C(12), C(6), 4, None, A.logical_shift_right)   # b0
                ts(V, C(13), C(12), 1, None, A.add)
                ts(V, C(13), C(13), 7, None, A.bitwise_and)          # b1
                ts(V, C(14), C(12), 2, None, A.add)
                ts(V, C(14), C(14), 7, None, A.bitwise_and)          # b2
                ts(V, C(9), C(0), 127, None, A.bitwise_and)          # w
                ts(V, C(10), C(9), 112, None, A.add)                 # w0 (pre-mask)
                w0_inst = ts(V, C(10), C(10), 127, None, A.bitwise_and)  # w0
                s16_inst = ts(V, C(11), C(6), 15, None, A.bitwise_and)   # s16
                for t in range(6):  # bvol[t] = b_{t//2} + vol*512
                    ts(V, C(2 + t), C(12 + t // 2), (2 * s + t % 2) * 512, None, A.add)

                # ---- gather index build: broadcast the 7 scalars
                # [d56, bvol0..5] to all partitions: replicate into a row
                # 32x, row->partition DMA, then two 32-aligned copies ----
                rep = spool.tile([1, 224], I32, tag="rep")
                r224 = rep[0:1, :].rearrange("p (a b) -> p a b", b=7)
                nc.vector.tensor_copy(
                    r224,
                    scal[0:1, cb + 1 : cb + 8]
                    .rearrange("p (a b) -> p a b", a=1)
                    .broadcast_to((1, 32, 7)),
                )
                bc7 = spool.tile([128, 7], I32, tag="bc7")
                nc.sync.dma_start(bc7[0:32, :], r224)
                nc.vector.tensor_copy(bc7[32:64, :], bc7[0:32, :])
                nc.vector.tensor_copy(bc7[64:128, :], bc7[0:64, :])
                dterm = spool.tile([128, 1], I32, tag="dterm")
                V.tensor_tensor(out=dterm[:, :], in0=iota16[:, :], in1=bc7[:, 0:1], op=A.add)
                ts(V, dterm[:, :], dterm[:, :], 63, 3, A.bitwise_and, A.logical_shift_left)
                idx32 = spool.tile([128, 6], I32, tag="idx32")
                for t in range(6):
                    V.tensor_tensor(
                        out=idx32[:, t : t + 1], in0=dterm[:, :],
                        in1=bc7[:, 1 + t : 2 + t], op=A.add,
                    )
                idx16 = spool.tile([128, 6], I16, tag="idx16")
                V.tensor_copy(idx16[:, :], idx32[:, :])

                # 96 x 8KB gather: three 16-row h-blocks per (d, c)
                Gt = gpool.tile([128, 2048], F32, tag="G")
                nc.gpsimd.dma_gather(
                    out_ap=Gt[:, :].rearrange("p (a b) -> p a b", a=1),
                    in_ap=xrows,
                    idxs_ap=idx16[:, :],
                    num_idxs=96,
                    num_idxs_reg=96,
                    elem_size=2048,
                )
                pending.append((s, Gt, cb, s16_inst, w0_inst))

            emit_extract(*pending.pop(0))

    nc.compile()
    return nc


def get_nc():
    if "nc" not in _cache:
        _cache["nc"] = _build()
    return _cache["nc"]


def kernel(x: np.ndarray, **run_kwargs) -> np.ndarray:
    assert x.shape == (32, 2, 64, 128, 128) and x.dtype == np.float32
    nc = get_nc()
    in_maps = []
    for c in range(N_CORES):
        xc = x[c * S_PER_CORE : (c + 1) * S_PER_CORE]           # [4, 2, 64, 128, 128]
        xc = np.ascontiguousarray(xc).reshape(N_VOLS, 128, FREE)
        in_maps.append({"x": xc})
    res = run_bass_kernel_spmd(nc, in_maps, core_ids=list(range(N_CORES)), **run_kwargs)
    out = np.empty((32, 2, 16, 32, 32), dtype=np.float32)
    for c in range(N_CORES):
        yc = res.results[c]["y"].reshape(S_PER_CORE, 2, 16, 32, 32)
        out[c * S_PER_CORE : (c + 1) * S_PER_CORE] = yc
    if run_kwargs:
        return out, res
    return out
